# revision 1
# baseline (speedup 1.0000x reference)
import sys

if "/opt/trn_rl_repo" not in sys.path:
    sys.path.insert(0, "/opt/trn_rl_repo")

import numpy as np
import ml_dtypes
from contextlib import ExitStack

import concourse.bass as bass
import concourse.tile as tile
from concourse import bacc, mybir
from concourse.bass_utils import run_bass_kernel_spmd

BF16 = ml_dtypes.bfloat16

B = 8192
C = 4096
KTAP = 5
HALF = KTAP // 2
N_CORES = 8
B_SH = B // N_CORES          # 1024
NCH = 512                    # one PSUM bank of fp32
NCHUNKS = B_SH // NCH        # 2


def _block_layout():
    outs, ms, ins = [], [], []
    outs.append(0); ms.append(126); ins.append(0)
    c0 = 126
    while c0 + 124 <= C - 126:
        outs.append(c0); ms.append(124); ins.append(c0 - 2)
        c0 += 124
    outs.append(c0); ms.append(C - c0); ins.append(C - 128)
    return outs, ms, ins

OUT_STARTS, M_LIST, IN_STARTS = _block_layout()
NBLK = len(OUT_STARTS)
assert sum(M_LIST) == C and NBLK == 33 and M_LIST[-1] == 126
NPAIR = (NBLK + 1) // 2  # 17 (last pair is a single block)

_COMPILED = None


def _build_program(repeat=1, barrier=False):
    # Three DMA-capable queues (SP, Act, gpsimd) share the ~56 us of DMA
    # stream time; PSUM drains (1x only) split across DVE (majority) and
    # Act; PE does matmuls.  Per-queue op order is monotone in block
    # index so the in-order engine queues never head-of-line block.
    nc = bacc.Bacc(
        "TRN2",
        target_bir_lowering=False,
        debug=False,
        enable_asserts=False,
    )
    f32 = mybir.dt.float32
    bf16 = mybir.dt.bfloat16
    xT_t = nc.dram_tensor("xT", [C, B_SH], bf16, kind="ExternalInput").ap()
    A_t = nc.dram_tensor("A", [128, C], bf16, kind="ExternalInput").ap()
    bc_t = nc.dram_tensor("bc", [128, NBLK], f32, kind="ExternalInput").ap()
    o_t = nc.dram_tensor("outP", [128, NBLK * B_SH], bf16, kind="ExternalOutput").ap()

    # loads: SP 15, Pool 16 (issued with lookahead so SWDGE latency is
    # hidden), Act 2.  Block 0/1 on SP/Act (HWDGE, fast completion).
    def q_load(blk):
        if blk < 2:
            return (nc.sync, nc.scalar)[blk]
        r = blk % 2
        if blk % 16 == 9:
            return nc.scalar
        return nc.sync if r == 0 else nc.gpsimd

    # stores: SP 12, Act 7, Pool 14 — emitted LAG blocks behind compute
    # so an in-order queue never parks a store ahead of ready loads.
    def q_store(blk):
        r = blk % 33
        if r % 5 in (1, 3):
            return nc.sync if r % 10 < 5 else nc.scalar
        return nc.gpsimd

    STORE_LAG = 6

    # drains: DVE for blk%5 in {0,2,4} (20), Act rest (13)
    def drain_on_dve(blk):
        return blk % 5 in (0, 2, 4)

    # A_tile in 4 independent chunks aligned to block boundaries
    A_EDGES = [OUT_STARTS[0], OUT_STARTS[8], OUT_STARTS[16], OUT_STARTS[24], C]

    def emit_once(inner_repeat):
        with tile.TileContext(nc) as tc:
            with ExitStack() as ctx:
                const_pool = ctx.enter_context(tc.tile_pool(name="const", bufs=1))
                x_pool = ctx.enter_context(tc.tile_pool(name="x", bufs=12))
                ps_pool = ctx.enter_context(
                    tc.tile_pool(name="ps", bufs=4, space="PSUM")
                )
                o_pool = ctx.enter_context(tc.tile_pool(name="o", bufs=10))

                A_tiles = [
                    const_pool.tile(
                        [128, A_EDGES[i + 1] - A_EDGES[i]], bf16, name=f"A_c{i}"
                    )
                    for i in range(4)
                ]
                bc_tile = const_pool.tile([128, NBLK], f32)
                nc.scalar.dma_start(bc_tile[:], bc_t[:])
                nc.scalar.dma_start(A_tiles[1][:], A_t[:, A_EDGES[1] : A_EDGES[2]])
                nc.gpsimd.dma_start(A_tiles[2][:], A_t[:, A_EDGES[2] : A_EDGES[3]])
                # (A_c2 stays on Pool: issued at head, needed only from block 16)

                def a_slice(blk, os_, m):
                    ci = min(blk // 8, 3)
                    off = os_ - A_EDGES[ci]
                    assert 0 <= off and off + m <= A_EDGES[ci + 1] - A_EDGES[ci]
                    return A_tiles[ci][:, off : off + m]

                def body():
                    ots = {}

                    def emit_store(b):
                        os_, m = OUT_STARTS[b], M_LIST[b]
                        q_store(b).dma_start(
                            o_t[0:m, bass.ds(b * B_SH, B_SH)], ots.pop(b)[:m, :]
                        )

                    for blk in range(NBLK):
                        os_, m, is_ = OUT_STARTS[blk], M_LIST[blk], IN_STARTS[blk]
                        xt = x_pool.tile([128, B_SH], bf16)
                        q_load(blk).dma_start(xt[:], xT_t[is_ : is_ + 128, :])
                        if blk == 0:
                            nc.sync.dma_start(
                                A_tiles[0][:], A_t[:, A_EDGES[0] : A_EDGES[1]]
                            )
                        if blk == 12:
                            nc.sync.dma_start(
                                A_tiles[3][:], A_t[:, A_EDGES[3] : A_EDGES[4]]
                            )
                        ps = ps_pool.tile([128, B_SH], f32)
                        for ch in range(NCHUNKS):
                            nc.tensor.matmul(
                                ps[:m, bass.ds(ch * NCH, NCH)],
                                a_slice(blk, os_, m),
                                xt[:, bass.ds(ch * NCH, NCH)],
                                start=True,
                                stop=True,
                            )
                        ot = o_pool.tile([128, B_SH], bf16)
                        bias_ap = bc_tile[0:m, blk : blk + 1]
                        if drain_on_dve(blk):
                            nc.vector.tensor_scalar_add(ot[:m, :], ps[:m, :], bias_ap)
                        else:
                            nc.scalar.add(ot[:m, :], ps[:m, :], bias_ap)
                        ots[blk] = ot
                        if blk >= STORE_LAG:
                            emit_store(blk - STORE_LAG)
                    for b in range(NBLK - STORE_LAG, NBLK):
                        emit_store(b)

                for _ in range(inner_repeat):
                    body()

    if barrier:
        for _ in range(repeat):
            emit_once(1)
    else:
        emit_once(repeat)

    nc.compile()
    return nc


def _host_prep(x, W, b):
    xT = x.T
    A_all = np.zeros((128, C), dtype=np.float32)
    cs = np.arange(C)
    blk_of_c = np.zeros(C, dtype=np.int64)
    for blk in range(NBLK):
        blk_of_c[OUT_STARTS[blk] : OUT_STARTS[blk] + M_LIST[blk]] = blk
    in_start_of_c = np.array(IN_STARTS)[blk_of_c]
    for t in range(KTAP):
        cin = cs + t - HALF
        valid = (cin >= 0) & (cin < C)
        j = cin - in_start_of_c
        valid &= (j >= 0) & (j < 128)
        A_all[j[valid], cs[valid]] = W[cs[valid], t]
    bcols = np.zeros((128, NBLK), dtype=np.float32)
    for blk in range(NBLK):
        os_, m = OUT_STARTS[blk], M_LIST[blk]
        bcols[:m, blk] = b[os_ : os_ + m]
    return xT, A_all.astype(BF16), bcols


def _run(x, W, b, trace=False, trace_kwargs=None):
    global _COMPILED
    if _COMPILED is None:
        _COMPILED = _build_program()
    nc = _COMPILED

    x = np.asarray(x, dtype=np.float32)
    W = np.asarray(W, dtype=np.float32)
    b = np.asarray(b, dtype=np.float32)
    xT, A_all, bcols = _host_prep(x, W, b)

    in_maps = []
    for i in range(N_CORES):
        shard = np.ascontiguousarray(xT[:, i * B_SH : (i + 1) * B_SH]).astype(BF16)
        in_maps.append({"xT": shard, "A": A_all, "bc": bcols})

    res = run_bass_kernel_spmd(
        nc,
        in_maps,
        core_ids=list(range(N_CORES)),
        trace=trace,
        **(trace_kwargs or {}),
    )
    outT = np.empty((C, B), dtype=np.float32)
    for i in range(N_CORES):
        outP = res.results[i]["outP"].reshape(128, NBLK, B_SH)
        for blk in range(NBLK):
            os_, m = OUT_STARTS[blk], M_LIST[blk]
            outT[os_ : os_ + m, i * B_SH : (i + 1) * B_SH] = outP[:m, blk, :].astype(
                np.float32
            )
    out = np.ascontiguousarray(outT.T)
    return out, res


def kernel(x, W, b):
    out, _ = _run(x, W, b, trace=False)
    return out

